# revision 1
# baseline (speedup 1.0000x reference)
"""Trainium2 Bass kernel for nn_DCTBranch (grayscale -> 8x8 DCT -> MLP -> resize).

Math: the whole front half of the pipeline is linear per 8x8 patch:
    feats = (D4 (x) D4) @ vec(patch_gray),  patch_gray = sum_c wc[c] * patch_c
    h     = relu(feats @ W1 + b1)
    emb   = h @ W2 + b2
so grayscale + DCT + W1 fold into one weight  W1''[(c,i,j), m] = wc[c] *
(kron(D4,D4).T @ W1)[(i,j), m], and the kernel is just two matmul layers over
8x8 patches.

Layout trick ("shifted-j"): the image is loaded in its natural row layout with
partitions p = 32*q + 8*c + i  (q = vertical quarter of the image, c = channel,
i = row-within-patch) and free dim (patch-row-in-quarter, w).  For each
within-patch column offset j (8 of them) the contraction over (c,i) [K=24] is
done with a strided rhs access pattern, accumulating the 8 j-matmuls into the
same PSUM bank.  The four image quarters run concurrently in the four 32-row
PE row-groups via tile_position, recovering 4x PE throughput.

Sharding: pure data parallel - batch 32 is split 4 images per core across the
8 NeuronCores; all weights replicated.
"""

import os
import numpy as np

B, C, H, W = 32, 3, 512, 512
N_CORES = 8
B_LOCAL = B // N_CORES

LAST_RESULTS = None  # BassKernelResults of the most recent run (for test.py)

_CACHE = {}


def _fold_weights(W1, b1, W2, b2):
    """Host-side fold of grayscale + DCT into W1; returns device-layout arrays."""
    import ml_dtypes

    PS = 8
    n = np.arange(PS)
    D = np.cos(np.pi * (2 * n[None, :] + 1) * n[:, None] / (2 * PS))
    D[0] *= 1.0 / np.sqrt(2.0)
    D *= np.sqrt(2.0 / PS)
    D4 = D[:4]  # [4,8] float64
    T = np.einsum("ki,lj->klij", D4, D4).reshape(16, 64)  # [16,64]
    W64 = T.T @ W1.astype(np.float64)  # [64,256]
    wc = np.array([0.299, 0.587, 0.114], np.float64)
    W1pp = np.concatenate([w * W64 for w in wc], axis=0)  # [192,256]; idx c*64+i*8+j

    # w1p[32*q + (c*8+i), j*256 + m] = W1pp[(c*8+i)*8 + j, m], replicated over q
    w1p = np.zeros((128, 2048), np.float32)
    blk = W1pp.reshape(24, 8, 256).transpose(0, 2, 1)  # [ci, m, j] -> want [ci, j*256+m]
    blk = np.ascontiguousarray(W1pp.reshape(24, 8, 256))  # [ci, j, m]
    for q in range(4):
        w1p[32 * q : 32 * q + 24, :] = blk.reshape(24, 2048)
    w1p = w1p.astype(ml_dtypes.bfloat16)

    # w2t[p, kt*64 + e] = W2[kt*128 + p, e]
    w2t = np.zeros((128, 128), np.float32)
    w2t[:, 0:64] = W2[0:128, :]
    w2t[:, 64:128] = W2[128:256, :]
    w2t = w2t.astype(ml_dtypes.bfloat16)

    b1d = np.ascontiguousarray(b1.reshape(2, 128).T.astype(np.float32))  # [128,2]
    b2d = np.ascontiguousarray(np.concatenate([b2, b2]).reshape(128, 1).astype(np.float32))  # [128,1] (both column-group halves)
    return w1p, w2t, b1d, b2d


def _build(b_local=B_LOCAL, use_tile_pos=True, reps=1):
    import bass_rust
    import concourse.bass as bass
    import concourse.tile as tile
    from concourse import bacc, mybir

    f32 = mybir.dt.float32
    bf16 = mybir.dt.bfloat16
    RELU = mybir.ActivationFunctionType.Relu

    nc = bacc.Bacc("TRN2", target_bir_lowering=False, debug=False)

    x_dram = nc.dram_tensor("x_shard", [b_local, 3, 512, 512], f32, kind="ExternalInput")
    w1_dram = nc.dram_tensor("w1p", [128, 2048], bf16, kind="ExternalInput")
    w2_dram = nc.dram_tensor("w2t", [128, 128], bf16, kind="ExternalInput")
    b1_dram = nc.dram_tensor("b1d", [128, 2], f32, kind="ExternalInput")
    b2_dram = nc.dram_tensor("b2d", [128, 1], f32, kind="ExternalInput")
    out_dram = nc.dram_tensor("out", [b_local, 64, 64, 64], f32, kind="ExternalOutput")

    with tile.TileContext(nc) as tc:
        with (
            tc.tile_pool(name="wpool", bufs=1) as wpool,
            tc.tile_pool(name="xpool", bufs=3) as xpool,
            tc.tile_pool(name="hpool", bufs=2) as hpool,
            tc.tile_pool(name="opool", bufs=2) as opool,
            tc.tile_pool(name="ps1", bufs=6, space="PSUM") as ps1,
            tc.tile_pool(name="ps2", bufs=2, space="PSUM") as ps2,
        ):
            w1_sb = wpool.tile([128, 2048], bf16)
            nc.sync.dma_start(w1_sb[:], w1_dram[:])
            w2_sb = wpool.tile([128, 128], bf16)
            nc.sync.dma_start(w2_sb[:], w2_dram[:])
            b1_sb = wpool.tile([128, 2], f32)
            nc.sync.dma_start(b1_sb[:], b1_dram[:])
            b2_sb = wpool.tile([128, 1], f32)
            nc.sync.dma_start(b2_sb[:], b2_dram[:])

            prev_mm = None
            for rep in range(reps):
              for b in range(b_local):
                prev_mm = None
                h_t = hpool.tile([128, 8192], bf16, name=f"h_{rep}_{b}", tag="h")  # [hidden%128, mt*4096 + n]
                # rows of x[b]: 512 = q(4) * g(16) * i(8)
                src_all = x_dram[b].rearrange(
                    "c (q g i) w -> q c i g w", q=4, g=16, i=8
                )
                xh = xpool.tile([128, 8192], bf16, name=f"xh_{rep}_{b}", tag="xh")  # [32q+8c+i, g*512 + w]
                xh_r = xh[:].rearrange("(q a) (g w) -> q a g w", q=4, w=512)
                for q in range(4):
                    for c in range(3):
                        nc.gpsimd.dma_start(
                            xh_r[q, 8 * c : 8 * (c + 1)], src_all[q, c]
                        )  # SWDGE: casts f32 -> bf16 in flight
                xq = xh[:].rearrange("p (g u j) -> p g u j", g=16, j=8)
                for mt in range(2):
                    for ns in range(2):
                        psums = [ps1.tile([128, 512], f32, name=f"acc{rep}_{q}_{b}_{ns}_{mt}", tag="acc") for q in range(4)]
                        for j in range(8):
                            for q in range(4):
                                lo = (j * 2 + mt) * 128
                                lhsT = w1_sb[32 * q : 32 * q + 24, lo : lo + 128]
                                rhs = xq[32 * q : 32 * q + 24, 8 * ns : 8 * ns + 8, :, j]
                                mm = nc.tensor.matmul(
                                    psums[q][:],
                                    lhsT,
                                    rhs,
                                    start=(j == 0),
                                    stop=(j == 7),
                                    tile_position=(32 * q, 0) if use_tile_pos else None,
                                )
                                # Force the scheduler to keep the j-major,
                                # row-group-interleaved order: consecutive PE
                                # matmuls hit different 32-row groups, so the
                                # next group's LDWEIGHTS overlaps the current
                                # group's MATMUL and the MMs themselves run
                                # concurrently in the array.
                                if prev_mm is not None:
                                    bass_rust.add_dep_helper(
                                        mm.ins, prev_mm.ins, sync=False,
                                        reason="pe row-group interleave order",
                                    )
                                prev_mm = mm
                        for q in range(4):
                            n0 = mt * 4096 + q * 1024 + ns * 512
                            nc.scalar.activation(
                                h_t[:, n0 : n0 + 512],
                                psums[q][:],
                                RELU,
                                bias=b1_sb[:, mt : mt + 1],
                                scale=1.0,
                            )
                # MLP2: pack chunk pairs into the two 64-partition column
                # groups of one PSUM bank so the two matmuls run concurrently.
                o_t = opool.tile([128, 2048], f32, name=f"o_{rep}_{b}", tag="o")
                for k in range(4):
                    p2 = ps2.tile([128, 512], f32, name=f"p2_{rep}_{b}_{k}", tag="p2")
                    for kt in range(2):
                        for par in range(2):
                            ch = 2 * k + par
                            mm = nc.tensor.matmul(
                                p2[64 * par : 64 * par + 64, :],
                                w2_sb[:, 64 * kt : 64 * kt + 64],
                                h_t[:, kt * 4096 + ch * 512 : kt * 4096 + (ch + 1) * 512],
                                start=(kt == 0), stop=(kt == 1),
                                tile_position=(0, 64 * par),
                                skip_group_check=True,
                            )
                            if prev_mm is not None:
                                bass_rust.add_dep_helper(
                                    mm.ins, prev_mm.ins, sync=False,
                                    reason="pe order mlp2",
                                )
                            prev_mm = mm
                    nc.vector.tensor_scalar_add(
                        o_t[:, k * 512 : (k + 1) * 512], p2[:], b2_sb[:, 0:1]
                    )
                # out n-index: n = gh*64+gw = (2k+par)*512 + s
                out_r = out_dram[b].rearrange(
                    "e (k par g2) gw -> par e k (g2 gw)", k=4, par=2, g2=8
                )
                o_r = o_t[:].rearrange("(par e) (k s) -> par e k s", par=2, s=512)
                for par in range(2):
                    nc.sync.dma_start(out_r[par], o_r[par])

    nc.compile()
    return nc


def kernel(x, W1, b1, W2, b2):
    global LAST_RESULTS
    from concourse.bass_utils import run_bass_kernel_spmd

    x = np.ascontiguousarray(np.asarray(x, dtype=np.float32))
    w1p, w2t, b1d, b2d = _fold_weights(
        np.asarray(W1), np.asarray(b1), np.asarray(W2), np.asarray(b2)
    )

    if "nc" not in _CACHE:
        _CACHE["nc"] = _build()
    nc = _CACHE["nc"]

    in_maps = []
    for core in range(N_CORES):
        in_maps.append(
            {
                "x_shard": np.ascontiguousarray(
                    x[core * B_LOCAL : (core + 1) * B_LOCAL]
                ),
                "w1p": w1p,
                "w2t": w2t,
                "b1d": b1d,
                "b2d": b2d,
            }
        )

    res = run_bass_kernel_spmd(
        nc,
        in_maps,
        core_ids=list(range(N_CORES)),
        trace=bool(os.environ.get("BASS_TRACE")),
    )
    LAST_RESULTS = res
    out = np.concatenate([res.results[i]["out"] for i in range(N_CORES)], axis=0)
    return out.astype(np.float32)



# revision 2
# speedup vs baseline: 1.0563x; 1.0563x over previous
"""Trainium2 Bass kernel for nn_DCTBranch (grayscale -> 8x8 DCT -> MLP -> resize).

Math: the whole front half of the pipeline is linear per 8x8 patch:
    feats = (D4 (x) D4) @ vec(patch_gray),  patch_gray = sum_c wc[c] * patch_c
    h     = relu(feats @ W1 + b1)
    emb   = h @ W2 + b2
so grayscale + DCT + W1 fold into one weight  W1''[(c,i,j), m] = wc[c] *
(kron(D4,D4).T @ W1)[(i,j), m], and the kernel is just two matmul layers over
8x8 patches.

Layout: the host pre-permutes x to [b, q, c, i, j, g, u] (q = vertical image
quarter, (i, j) = position within 8x8 patch, g = patch-row within quarter,
u = patch-col).  Each (image, q) is then ONE fully contiguous 768 KB HBM
region that lands as one SWDGE (casting f32->bf16 in flight) DMA onto SBUF
partitions 32q + 8c + i with a contiguous 8192-element free dim (j,g,u).
For each within-patch column offset j the contraction over (c,i) [K=24] uses
a CONTIGUOUS 512-element rhs window, accumulating the 8 j-matmuls into the
same PSUM bank.  The four quarters run concurrently in the four 32-row PE
row-groups via tile_position.

Sharding: pure data parallel - batch 32 is split 4 images per core across the
8 NeuronCores; all weights replicated.
"""

import os
import numpy as np

B, C, H, W = 32, 3, 512, 512
N_CORES = 8
B_LOCAL = B // N_CORES

LAST_RESULTS = None  # BassKernelResults of the most recent run (for test.py)

_CACHE = {}


def _prep_x(x):
    """[B, 3, 512, 512] f32 -> [B, q4, c3, i8, j8, g16, u64] contiguous, flat
    to [B, 4, 24, 8192].  Pure permutation (dtype preserved)."""
    x = np.asarray(x, dtype=np.float32)
    xp = x.reshape(B, 3, 4, 16, 8, 64, 8)          # b c q g i u j
    xp = xp.transpose(0, 2, 1, 4, 6, 3, 5)         # b q c i j g u
    return np.ascontiguousarray(xp).reshape(B, 4, 24, 8192)


def _fold_weights(W1, b1, W2, b2):
    """Host-side fold of grayscale + DCT into W1; returns device-layout arrays."""
    import ml_dtypes

    PS = 8
    n = np.arange(PS)
    D = np.cos(np.pi * (2 * n[None, :] + 1) * n[:, None] / (2 * PS))
    D[0] *= 1.0 / np.sqrt(2.0)
    D *= np.sqrt(2.0 / PS)
    D4 = D[:4]  # [4,8] float64
    T = np.einsum("ki,lj->klij", D4, D4).reshape(16, 64)  # [16,64]
    W64 = T.T @ W1.astype(np.float64)  # [64,256]
    wc = np.array([0.299, 0.587, 0.114], np.float64)
    W1pp = np.concatenate([w * W64 for w in wc], axis=0)  # [192,256]; idx c*64+i*8+j
    # w1p[32*q + (c*8+i), j*256 + m] = W1pp[(c*8+i)*8 + j, m], replicated over q
    w1p = np.zeros((128, 2048), np.float32)
    blk = np.ascontiguousarray(W1pp.reshape(24, 8, 256))  # [ci, j, m]
    for q in range(4):
        w1p[32 * q : 32 * q + 24, :] = blk.reshape(24, 2048)
    w1p = w1p.astype(ml_dtypes.bfloat16)

    # w2t[p, kt*64 + e] = W2[kt*128 + p, e]
    w2t = np.zeros((128, 128), np.float32)
    w2t[:, 0:64] = W2[0:128, :]
    w2t[:, 64:128] = W2[128:256, :]
    w2t = w2t.astype(ml_dtypes.bfloat16)

    b1d = np.ascontiguousarray(b1.reshape(2, 128).T.astype(np.float32))  # [128,2]
    b2d = np.ascontiguousarray(np.concatenate([b2, b2]).reshape(128, 1).astype(np.float32))  # [128,1]
    return w1p, w2t, b1d, b2d


def _build(b_local=B_LOCAL, reps=1):
    import bass_rust
    import concourse.bass as bass
    import concourse.tile as tile
    from concourse import bacc, mybir

    f32 = mybir.dt.float32
    bf16 = mybir.dt.bfloat16
    RELU = mybir.ActivationFunctionType.Relu

    nc = bacc.Bacc("TRN2", target_bir_lowering=False, debug=False)

    x_dram = nc.dram_tensor("x_shard", [b_local, 4, 24, 8192], f32, kind="ExternalInput")
    w1_dram = nc.dram_tensor("w1p", [128, 2048], bf16, kind="ExternalInput")
    w2_dram = nc.dram_tensor("w2t", [128, 128], bf16, kind="ExternalInput")
    b1_dram = nc.dram_tensor("b1d", [128, 2], f32, kind="ExternalInput")
    b2_dram = nc.dram_tensor("b2d", [128, 1], f32, kind="ExternalInput")
    out_dram = nc.dram_tensor("out", [b_local, 64, 64, 64], f32, kind="ExternalOutput")

    with tile.TileContext(nc) as tc:
        with (
            tc.tile_pool(name="wpool", bufs=1) as wpool,
            tc.tile_pool(name="xpool", bufs=3) as xpool,
            tc.tile_pool(name="hpool", bufs=2) as hpool,
            tc.tile_pool(name="opool", bufs=2) as opool,
            tc.tile_pool(name="ps1", bufs=6, space="PSUM") as ps1,
            tc.tile_pool(name="ps2", bufs=2, space="PSUM") as ps2,
        ):
            w1_sb = wpool.tile([128, 2048], bf16)
            nc.sync.dma_start(w1_sb[:], w1_dram[:])
            w2_sb = wpool.tile([128, 128], bf16)
            nc.sync.dma_start(w2_sb[:], w2_dram[:])
            b1_sb = wpool.tile([128, 2], f32)
            nc.sync.dma_start(b1_sb[:], b1_dram[:])
            b2_sb = wpool.tile([128, 1], f32)
            nc.sync.dma_start(b2_sb[:], b2_dram[:])

            prev_mm = None
            for rep in range(reps):
              for b in range(b_local):
                prev_mm = None
                h_t = hpool.tile([128, 8192], bf16, name=f"h_{rep}_{b}", tag="h")
                xh = xpool.tile([128, 8192], bf16, name=f"xh_{rep}_{b}", tag="xh")
                # one contiguous DMA per image quarter: 24 partitions x 16KB
                xh_q = xh[:].rearrange("(q a) f -> q a f", q=4)
                for q in range(4):
                    nc.gpsimd.dma_start(xh_q[q, 0:24], x_dram[b, q])
                for mt in range(2):
                    for ns in range(2):
                        psums = [ps1.tile([128, 512], f32, name=f"acc{rep}_{q}_{b}_{ns}_{mt}", tag="acc") for q in range(4)]
                        for j in range(8):
                            for q in range(4):
                                lo = (j * 2 + mt) * 128
                                lhsT = w1_sb[32 * q : 32 * q + 24, lo : lo + 128]
                                rhs = xh[32 * q : 32 * q + 24,
                                         j * 1024 + ns * 512 : j * 1024 + ns * 512 + 512]
                                mm = nc.tensor.matmul(
                                    psums[q][:],
                                    lhsT,
                                    rhs,
                                    start=(j == 0),
                                    stop=(j == 7),
                                    tile_position=(32 * q, 0),
                                )
                                # Keep the j-major, row-group-interleaved order:
                                # consecutive PE matmuls hit different 32-row
                                # groups, so the next group's LDWEIGHTS overlaps
                                # the current group's MATMUL and the MMs run
                                # concurrently in the array.
                                if prev_mm is not None:
                                    bass_rust.add_dep_helper(
                                        mm.ins, prev_mm.ins, sync=False,
                                        reason="pe row-group interleave order",
                                    )
                                prev_mm = mm
                        for q in range(4):
                            n0 = mt * 4096 + q * 1024 + ns * 512
                            nc.scalar.activation(
                                h_t[:, n0 : n0 + 512],
                                psums[q][:],
                                RELU,
                                bias=b1_sb[:, mt : mt + 1],
                                scale=1.0,
                            )
                # MLP2: pack chunk pairs into the two 64-partition column
                # groups of one PSUM bank so the two matmuls run concurrently.
                o_t = opool.tile([128, 2048], f32, name=f"o_{rep}_{b}", tag="o")
                for k in range(4):
                    p2 = ps2.tile([128, 512], f32, name=f"p2_{rep}_{b}_{k}", tag="p2")
                    for kt in range(2):
                        for par in range(2):
                            ch = 2 * k + par
                            mm = nc.tensor.matmul(
                                p2[64 * par : 64 * par + 64, :],
                                w2_sb[:, 64 * kt : 64 * kt + 64],
                                h_t[:, kt * 4096 + ch * 512 : kt * 4096 + (ch + 1) * 512],
                                start=(kt == 0), stop=(kt == 1),
                                tile_position=(0, 64 * par),
                                skip_group_check=True,
                            )
                            if prev_mm is not None:
                                bass_rust.add_dep_helper(
                                    mm.ins, prev_mm.ins, sync=False,
                                    reason="pe order mlp2",
                                )
                            prev_mm = mm
                    nc.vector.tensor_scalar_add(
                        o_t[:, k * 512 : (k + 1) * 512], p2[:], b2_sb[:, 0:1]
                    )
                # out n-index: n = gh*64+gw = (2k+par)*512 + s
                out_r = out_dram[b].rearrange(
                    "e (k par g2) gw -> par e k (g2 gw)", k=4, par=2, g2=8
                )
                o_r = o_t[:].rearrange("(par e) (k s) -> par e k s", par=2, s=512)
                for par in range(2):
                    nc.sync.dma_start(out_r[par], o_r[par])

    nc.compile()
    return nc


def kernel(x, W1, b1, W2, b2):
    global LAST_RESULTS
    from concourse.bass_utils import run_bass_kernel_spmd

    xp = _prep_x(x)
    w1p, w2t, b1d, b2d = _fold_weights(
        np.asarray(W1), np.asarray(b1), np.asarray(W2), np.asarray(b2)
    )

    if "nc" not in _CACHE:
        _CACHE["nc"] = _build()
    nc = _CACHE["nc"]

    in_maps = []
    for core in range(N_CORES):
        in_maps.append(
            {
                "x_shard": np.ascontiguousarray(
                    xp[core * B_LOCAL : (core + 1) * B_LOCAL]
                ),
                "w1p": w1p,
                "w2t": w2t,
                "b1d": b1d,
                "b2d": b2d,
            }
        )

    res = run_bass_kernel_spmd(
        nc,
        in_maps,
        core_ids=list(range(N_CORES)),
        trace=bool(os.environ.get("BASS_TRACE")),
    )
    LAST_RESULTS = res
    out = np.concatenate([res.results[i]["out"] for i in range(N_CORES)], axis=0)
    return out.astype(np.float32)
